# revision 47
# baseline (speedup 1.0000x reference)
"""Trainium2 Bass kernel for nn_Attention_35588099015470.

Full transformer attention block: LoRA linears (folded host-side) + RoPE +
causal SDPA + output projection, B=2 T=2048 C=2048 H=16 D=128, fp32 in/out.

Sharding: tensor-parallel over heads - 8 cores x 2 heads, AllToAll to
token-parallel for the output projection. Mixed-precision datapath:

 - q/k/v projections run in fp8e4m3 with DoubleRow perf mode (two 128-deep
   contraction chunks per pass = 2x MAC throughput), EXCEPT the first 512
   tokens of each batch which run in bf16: those tokens have small attention
   fan-in, so quantization errors there don't average out and dominate the
   max-error metric.
 - scores (QK^T) are bf16 (fp8 DR doesn't help at D=128 contraction).
 - softmax probabilities and PV run fp8+DoubleRow for query tiles >= 1;
   query tile 0 (tokens 0-511) runs bf16 against a bf16 copy of v.
 - output projection is bf16 (y values for early tokens are large and fp8
   there busts the error budget), split per batch: batch 0 projects in the
   shadow of the last attention block, batch 1 in two contraction stages so
   its first half overlaps the final AllToAll.
 - q/k/v activations stay SBUF-resident between phases, collectives and the
   projection weights are bf16, projection weights prefetch during attention,
   and all host tensors are tile-blocked so DMA moves 4-16KB per partition
   per transfer instead of 512B packets.
 - softmax row sums come from a ones-matmul folded into the fp8/bf16 PV
   stream; normalization is per-query-tile and eager so each AllToAll
   launches as soon as its head finishes.

Scale bookkeeping: x is scaled by XS=32 and weights by WS=2048 before fp8
quantization (keeps values clear of subnormals, max < 240); the 1/(XS*WS)
descale folds into the RoPE cos/sin tables for q/k and into the PSUM->SBUF
copy for v (which carries an extra VS=16 so fp8 v has headroom). exp() gets
a -ln(8) bias so unnormalized probabilities stay < 240 (fp8 max); the
normalization cancels both the bias and VS (the row-sum ones matmul uses VS
as its constant value).

Biases are guaranteed zero by the problem's setup_inputs and the mask is the
causal tril; if either assumption is violated at runtime we fall back to a
host reference implementation so the kernel stays correct on any input.
"""
import sys

sys.path.insert(0, "/opt/trn_rl_repo")

import numpy as np
import ml_dtypes
from contextlib import ExitStack

import concourse.tile as tile
from concourse import bacc, mybir
from concourse.bass_utils import run_bass_kernel_spmd

dt = mybir.dt
F8 = dt.float8e4
BF = dt.bfloat16
DR = mybir.MatmulPerfMode.DoubleRow

B, T, C, H, R = 2, 2048, 2048, 16, 8
D = C // H            # 128
NCORES = 8
HPC = H // NCORES     # heads per core = 2
P = 128
TT = (B * T) // 512   # 8 token tiles of 512
KC = C // P           # 16 contraction chunks
QT = T // 512         # 4 query tiles per (b, h)
SCALE = 1.0 / float(np.sqrt(D))

XS = 32.0             # x fp8 scale
WS = 2048.0           # weight fp8 scale
VS = 16.0             # v fp8 scale
EXP_BIAS = float(np.log(0.125))   # keeps exp output < 240 (fp8 max)
A8SC = VS / (XS * WS)             # fp8 v psum -> sbuf copy scale

BF_TT = (0, 4)        # token tiles computed in bf16 (first 512 tokens/batch)
TT_ORDER = [1, 2, 3, 5, 6, 7, 0, 4]   # fp8 tiles first (smaller first DMAs)

_PROGRAM = None


def _build_program():
    nc = bacc.Bacc("TRN2", target_bir_lowering=False, debug=False,
                   num_devices=NCORES)

    # tile-blocked inputs: [..., P, free] with per-partition-contiguous runs
    x8_d = nc.dram_tensor("x8b", [TT, P, KC * 512], F8, kind="ExternalInput")
    xb_d = nc.dram_tensor("xbb", [B, P, KC * 512], BF, kind="ExternalInput")
    w8_d = {nm: nc.dram_tensor(f"w8{nm}", [P, KC * HPC * D], F8,
                               kind="ExternalInput") for nm in ("q", "k", "v")}
    wb_d = {nm: nc.dram_tensor(f"wb{nm}", [P, KC * HPC * D], BF,
                               kind="ExternalInput") for nm in ("q", "k", "v")}
    pwB_d = nc.dram_tensor("pwB", [KC, P, KC, P], BF, kind="ExternalInput")
    cosA_d = nc.dram_tensor("cosA", [P, B * T], dt.float32, kind="ExternalInput")
    sinA_d = nc.dram_tensor("sinA", [P, B * T], dt.float32, kind="ExternalInput")
    cosAs_d = nc.dram_tensor("cosAs", [P, B * T], dt.float32, kind="ExternalInput")
    sinAs_d = nc.dram_tensor("sinAs", [P, B * T], dt.float32, kind="ExternalInput")
    lstep_d = nc.dram_tensor("lstep", [P, P], BF, kind="ExternalInput")
    rmask_d = nc.dram_tensor("rmask", [4, P, 512], BF, kind="ExternalInput")

    outT_d = nc.dram_tensor("outT", [C, 512], BF, kind="ExternalOutput")

    with tile.TileContext(nc) as tc, ExitStack() as ctx:
        dram = ctx.enter_context(tc.tile_pool(name="dram", bufs=1, space="DRAM"))
        # A2A staging: one collective per (batch, head-local), bf16
        chs = [[dram.tile([NCORES, D, 256], BF, name=f"ch_{b}_{hl}")
                for hl in range(HPC)] for b in range(B)]
        yos = [[dram.tile([NCORES * D, 256], BF, name=f"yo_{b}_{hl}")
                for hl in range(HPC)] for b in range(B)]

        # persistent SBUF (survives all phases)
        cst = ctx.enter_context(tc.tile_pool(name="cst", bufs=1))
        qT_sb = cst.tile([P, HPC, B * T], BF, name="qT_sb")
        kT_sb = cst.tile([P, HPC, B * T], BF, name="kT_sb")
        v8_sb = cst.tile([P, TT * 4, HPC * D], F8, name="v8_sb")
        v0_sb = cst.tile([P, B * 4, HPC * D], BF, name="v0_sb")

        ones_bf = cst.tile([P, P], BF, name="ones_bf")
        nc.any.memset(ones_bf[:], VS)
        ones8 = cst.tile([P, 2, P], F8, name="ones8")
        nc.any.memset(ones8[:], VS)
        lstep = cst.tile([P, P], BF, name="lstep")
        rmask = cst.tile([P, 4, 512], BF, name="rmask")
        ebias = cst.tile([P, 1], dt.float32, name="ebias")
        nc.any.memset(ebias[:], EXP_BIAS)

        # output-projection weights, prefetched in two halves: pw_lo during
        # phase A's DMA slack, pw_hi right after the first A2A's writes
        pwp = ctx.enter_context(tc.tile_pool(name="pw", bufs=1))
        pw_lo = pwp.tile([P, KC // 2, KC, P], BF, name="pw_lo")
        pw_hi = pwp.tile([P, KC // 2, KC, P], BF, name="pw_hi")

        # ---------------- Phase A: q/k/v projections + RoPE -----------------
        with tc.tile_pool(name="pa_w", bufs=1) as wp, \
             tc.tile_pool(name="pa_x8", bufs=3) as x8p, \
             tc.tile_pool(name="pa_xb", bufs=1) as xbp, \
             tc.tile_pool(name="pa_cs", bufs=2) as csp, \
             tc.tile_pool(name="pa_tmp", bufs=2) as tp, \
             tc.tile_pool(name="pa_ps", bufs=1, space="PSUM") as pp:

            # DMA emission order = queue processing order: first fp8 tile and
            # fp8 weights first (each split 4 ways across queues), then the
            # second tile, then the big bf16 sets
            def load4(dst3, src2, inner):
                # dst3 [P, KC, inner] <- src2 [P, KC*inner], 4 parallel DMAs
                for g in range(4):
                    nc.sync.dma_start(
                        dst3[:, g * 4:(g + 1) * 4, :],
                        src2[:, g * 4 * inner:(g + 1) * 4 * inner])

            xts = {}
            xt = x8p.tile([P, KC, 512], F8, name=f"xt8_{TT_ORDER[0]}", tag="x8")
            load4(xt, x8_d.ap()[TT_ORDER[0]], 512)
            xts[TT_ORDER[0]] = xt
            w8s = {}
            for nm in ("q", "k", "v"):
                w_sb = wp.tile([P, KC, HPC * D], F8, name=f"w8{nm}_sb")
                load4(w_sb, w8_d[nm].ap(), HPC * D)
                w8s[nm] = w_sb
            xt = x8p.tile([P, KC, 512], F8, name=f"xt8_{TT_ORDER[1]}", tag="x8")
            load4(xt, x8_d.ap()[TT_ORDER[1]], 512)
            xts[TT_ORDER[1]] = xt
            cs_first = {}
            tt0 = TT_ORDER[0]
            for tag, dsrc in (("csc", cosAs_d), ("css", sinAs_d)):
                t_ = csp.tile([P, 512], dt.float32, tag=tag, name=f"{tag}_{tt0}")
                nc.sync.dma_start(t_[:], dsrc.ap()[:, tt0 * 512:(tt0 + 1) * 512])
                cs_first[tag] = t_
            wbs = {}
            for nm in ("q", "k", "v"):
                w_sb = wp.tile([P, KC, HPC * D], BF, name=f"wb{nm}_sb")
                nc.sync.dma_start(w_sb[:], wb_d[nm].ap())
                wbs[nm] = w_sb
            nc.sync.dma_start(lstep[:], lstep_d.ap())
            for o in range(4):
                nc.sync.dma_start(rmask[:, o, :], rmask_d.ap()[o])

            xbts = {}
            for ti, tt in enumerate(TT_ORDER):
                tsl = slice(tt * 512, (tt + 1) * 512)
                bf = tt in BF_TT
                if ti == 1:
                    # bf16 x for tile 0 early, while tiles 1-2 compute
                    xb0 = xbp.tile([P, KC, 512], BF, name="xtb_0", tag="xb")
                    for g in range(8):
                        nc.sync.dma_start(
                            xb0[:, g * 2:(g + 1) * 2, :],
                            xb_d.ap()[0][:, g * 1024:(g + 1) * 1024])
                    xbts[0] = xb0
                if ti == 3:
                    # prefetch the projection weights in phase A's DMA slack
                    # (keeping them out of phase B's queues, where they would
                    # delay the A2A staging writes via coarse DMA semaphores)
                    for co in range(KC // 2):
                        nc.sync.dma_start(pw_lo[:, co, :, :], pwB_d.ap()[co])
                if ti == 4:
                    xb4 = xbp.tile([P, KC, 512], BF, name="xtb_4", tag="xb")
                    load4(xb4, xb_d.ap()[1], 512)
                    xbts[4] = xb4
                if ti == 5:
                    for co in range(KC // 2):
                        nc.sync.dma_start(pw_hi[:, co, :, :],
                                          pwB_d.ap()[KC // 2 + co])
                # fp8 x tiles prefetch two iterations ahead so the DMA
                # queue has a full tile of lead time
                if ti + 2 < len(TT_ORDER):
                    nxt = TT_ORDER[ti + 2]
                    if nxt not in BF_TT and nxt not in xts:
                        xt2 = x8p.tile([P, KC, 512], F8, name=f"xt8_{nxt}",
                                       tag="x8")
                        load4(xt2, x8_d.ap()[nxt], 512)
                        xts[nxt] = xt2
                if bf:
                    xt = xbts[tt]
                else:
                    xt = xts[tt]
                cd, sd = (cosA_d, sinA_d) if bf else (cosAs_d, sinAs_d)
                if ti == 0:
                    cs_c, cs_s = cs_first["csc"], cs_first["css"]
                else:
                    cs_c = csp.tile([P, 512], dt.float32, tag="csc",
                                    name=f"csc_{tt}")
                    nc.sync.dma_start(cs_c[:], cd.ap()[:, tsl])
                    cs_s = csp.tile([P, 512], dt.float32, tag="css",
                                    name=f"css_{tt}")
                    nc.sync.dma_start(cs_s[:], sd.ap()[:, tsl])

                for nm, dst in (("q", qT_sb), ("k", kT_sb)):
                    for mt in range(HPC):
                        ps = pp.tile([P, 512], dt.float32, tag="qk", bufs=4,
                                     name=f"psA_{tt}_{nm}_{mt}")
                        msl = slice(mt * P, (mt + 1) * P)
                        if bf:
                            w_sb = wbs[nm]
                            for kc in range(KC):
                                nc.tensor.matmul(
                                    ps[:], w_sb[:, kc, msl], xt[:, kc, :],
                                    start=(kc == 0), stop=(kc == KC - 1))
                        else:
                            w_sb = w8s[nm]
                            for c in range(KC // 2):
                                nc.tensor.matmul(
                                    ps[:], w_sb[:, 2 * c:2 * c + 2, msl],
                                    xt[:, 2 * c:2 * c + 2, :],
                                    start=(c == 0), stop=(c == KC // 2 - 1),
                                    perf_mode=DR)
                        # rope: y = raw*cosA + halfswap(raw)*sinA
                        # (cs tiles carry the fp8 descale for fp8 tiles)
                        t1 = tp.tile([P, 512], dt.float32, tag="t1",
                                     name=f"t1_{tt}_{nm}_{mt}")
                        nc.vector.tensor_mul(t1[:], ps[:], cs_c[:])
                        t2 = tp.tile([P, 512], dt.float32, tag="t2",
                                     name=f"t2_{tt}_{nm}_{mt}")
                        nc.vector.tensor_mul(t2[0:64, :], ps[64:128, :],
                                             cs_s[0:64, :])
                        nc.vector.tensor_mul(t2[64:128, :], ps[0:64, :],
                                             cs_s[64:128, :])
                        nc.vector.tensor_add(dst[:, mt, tsl], t1[:], t2[:])

                # v in natural [token, d] layout: x chunks stationary,
                # weight chunks moving -> out [128 tokens, 256 features]
                for tc_ in range(4):
                    psv = pp.tile([P, HPC * D], dt.float32, tag="vv", bufs=4,
                                  name=f"psV_{tt}_{tc_}")
                    tcs = slice(tc_ * P, (tc_ + 1) * P)
                    if bf:
                        w_sb = wbs["v"]
                        for kc in range(KC):
                            nc.tensor.matmul(
                                psv[:], xt[:, kc, tcs], w_sb[:, kc, :],
                                start=(kc == 0), stop=(kc == KC - 1))
                    else:
                        w_sb = w8s["v"]
                        for c in range(KC // 2):
                            nc.tensor.matmul(
                                psv[:], xt[:, 2 * c:2 * c + 2, tcs],
                                w_sb[:, 2 * c:2 * c + 2, :],
                                start=(c == 0), stop=(c == KC // 2 - 1),
                                perf_mode=DR)
                    sc = VS if bf else A8SC
                    nc.scalar.activation(v8_sb[:, tt * 4 + tc_, :], psv[:],
                                         mybir.ActivationFunctionType.Copy,
                                         scale=sc)
                    if bf:
                        nc.scalar.activation(
                            v0_sb[:, (tt // 4) * 4 + tc_, :], psv[:],
                            mybir.ActivationFunctionType.Copy, scale=VS)

        # ---------------- Phase B: causal attention per (b, head) ----------
        # + batch-0 output projection in the shadow of the last block
        ycp = ctx.enter_context(tc.tile_pool(name="yc", bufs=1))
        ocp = ctx.enter_context(tc.tile_pool(name="oc", bufs=3))

        def pw_co(co):
            return pw_lo[:, co, :, :] if co < KC // 2 else \
                pw_hi[:, co - KC // 2, :, :]

        ybs = {0: {}, 1: {}}

        def emit_gather(b, hl):
            yb = ycp.tile([P, NCORES, 256], BF, name=f"yb_{b}_{hl}")
            nc.sync.dma_start(
                yb[:], yos[b][hl][:].rearrange("(a p) t -> p a t", p=P))
            ybs[b][hl] = yb

        with tc.tile_pool(name="pb_p8", bufs=4) as p8p, \
             tc.tile_pool(name="pb_pb", bufs=3) as pbp, \
             tc.tile_pool(name="pb_pv", bufs=6) as pvp, \
             tc.tile_pool(name="pb_y", bufs=8) as yp, \
             tc.tile_pool(name="pb_ps", bufs=1, space="PSUM") as pb:

            def emit_phase_c(b):
                for co in range(KC):
                    pso = pb.tile([P, 256], dt.float32, tag="co", bufs=1,
                                  name=f"pso_{b}_{co}")
                    i = 0
                    for hl in range(HPC):
                        for r in range(NCORES):
                            nc.tensor.matmul(
                                pso[:], pw_co(co)[:, hl * 8 + r, :],
                                ybs[b][hl][:, r, :],
                                start=(i == 0), stop=(i == KC - 1))
                            i += 1
                    oo = ocp.tile([P, 256], BF, tag="oo",
                                  name=f"oo_{b}_{co}")
                    nc.scalar.copy(oo[:], pso[:])
                    nc.sync.dma_start(
                        outT_d.ap()[co * P:(co + 1) * P, b * 256:(b + 1) * 256],
                        oo[:])

            for b in range(B):
                for hl in range(HPC):
                    kT_h = kT_sb[:, hl, b * T:(b + 1) * T]
                    qT_h = qT_sb[:, hl, b * T:(b + 1) * T]
                    hsl = slice(hl * D, (hl + 1) * D)

                    for qt in range(QT):
                        qTt = qT_h[:, qt * 512:(qt + 1) * 512]
                        n = 4 * (qt + 1)
                        smps = pb.tile([P, 512], dt.float32, tag="sm", bufs=1,
                                       name=f"sm_{b}_{hl}_{qt}")
                        pvps = pb.tile([P, 512], dt.float32, tag="pv", bufs=2,
                                       name=f"pv_{b}_{hl}_{qt}")

                        sc_tiles = {}
                        npair = n // 2

                        def emit_sc_pair(c, _q=qTt, _k=kT_h, _n=n,
                                         _sc=sc_tiles, _b=b, _hl=hl, _qt=qt):
                            # score pair in one 2-bank psum tile so one exp
                            # covers both chunks (ACT has ~300ns fixed cost)
                            ps = pb.tile([P, 2, 512], dt.float32, tag="sc",
                                         bufs=2, name=f"sc_{_b}_{_hl}_{_qt}_{c}")
                            for half in range(2):
                                jc = 2 * c + half
                                diag = jc >= _n - 4
                                nc.tensor.matmul(
                                    ps[:, half, :],
                                    _k[:, jc * P:(jc + 1) * P], _q[:],
                                    start=True, stop=not diag)
                                if diag:
                                    o = jc - (_n - 4)
                                    nc.tensor.matmul(ps[:, half, :], lstep[:],
                                                     rmask[:, o, :],
                                                     start=False, stop=True)
                            _sc[c] = ps

                        emit_sc_pair(0)
                        if npair > 1:
                            emit_sc_pair(1)
                        for c in range(npair):
                            scps = sc_tiles.pop(c)
                            if qt == 0:
                                # bf16 path: accurate probabilities and v for
                                # the low-fan-in early tokens
                                pT = pbp.tile([P, 2, 512], BF, tag="pTb",
                                              name=f"pTb_{b}_{hl}_{c}")
                            else:
                                pT = p8p.tile([P, 2, 512], F8, tag="pT8",
                                              name=f"pT8_{b}_{hl}_{qt}_{c}")
                            nc.scalar.activation(
                                pT[:], scps[:],
                                mybir.ActivationFunctionType.Exp,
                                bias=ebias[:], scale=SCALE)
                            if c + 2 < npair:
                                emit_sc_pair(c + 2)
                            if qt == 0:
                                for half in range(2):
                                    jc = 2 * c + half
                                    nc.tensor.matmul(
                                        smps[:], ones_bf[:], pT[:, half, :],
                                        start=(jc == 0), stop=(jc == 3))
                                    nc.tensor.matmul(
                                        pvps[:], v0_sb[:, b * 4 + jc, hsl],
                                        pT[:, half, :],
                                        start=(jc == 0), stop=(jc == 3))
                            else:
                                nc.tensor.matmul(
                                    smps[:], ones8[:], pT[:],
                                    start=(c == 0), stop=(c == npair - 1),
                                    perf_mode=DR)
                                nc.tensor.matmul(
                                    pvps[:],
                                    v8_sb[:, b * 16 + 2 * c:b * 16 + 2 * c + 2,
                                          hsl],
                                    pT[:],
                                    start=(c == 0), stop=(c == npair - 1),
                                    perf_mode=DR)

                        # eager per-qt normalization: fast-approx reciprocal
                        # of the sum row, broadcast, scale PV
                        pv_sb = pvp.tile([P, 512], dt.float32, tag="pvsb",
                                         name=f"pvsb_{b}_{hl}_{qt}")
                        nc.vector.tensor_copy(pv_sb[:], pvps[:])
                        smrow = yp.tile([1, 512], dt.float32, tag="smrow",
                                        bufs=6, name=f"smrow_{b}_{hl}_{qt}")
                        nc.vector.tensor_copy(smrow[:], smps[0:1, :])
                        rrow = yp.tile([1, 512], dt.float32, tag="rrow",
                                       bufs=6, name=f"rrow_{b}_{hl}_{qt}")
                        rscr = yp.tile([1, 512], dt.float32, tag="rscr",
                                       bufs=6, name=f"rscr_{b}_{hl}_{qt}")
                        nc.vector.reciprocal_approx_accurate(
                            rrow[:], smrow[:], rscr[:])
                        bc = yp.tile([P, 512], dt.float32, tag="bc", bufs=4,
                                     name=f"bc_{b}_{hl}_{qt}")
                        nc.gpsimd.partition_broadcast(bc[:], rrow[:])
                        yt = yp.tile([P, 512], BF, tag="yt",
                                     name=f"yt_{b}_{hl}_{qt}")
                        nc.vector.tensor_mul(yt[:], pv_sb[:], bc[:])
                        nc.sync.dma_start(chs[b][hl][2 * qt][:, :],
                                          yt[:, 0:256])
                        nc.sync.dma_start(chs[b][hl][2 * qt + 1][:, :],
                                          yt[:, 256:512])

                    if not (b == 1 and hl == 1):
                        nc.gpsimd.collective_compute(
                            "AllToAll", mybir.AluOpType.bypass,
                            replica_groups=[list(range(NCORES))],
                            ins=[chs[b][hl].opt()], outs=[yos[b][hl].opt()],
                        )
                    if b == 1 and hl == 0:
                        emit_gather(1, 0)
                        emit_gather(0, 0)
                        emit_gather(0, 1)

            # ------------ Phase C: output projection, both batches ---------
            # all four attention blocks ran back-to-back; batch 0's data is
            # long delivered, so C-b0 runs while the (1,1) A2A (emitted below,
            # triggered from gpsimd as soon as its staging writes land) flies.
            # C-b1's first matmul is emitted after the (1,1) gather, so its
            # coarse DMA-semaphore wait covers exactly that gather.
            emit_phase_c(0)
            nc.gpsimd.collective_compute(
                "AllToAll", mybir.AluOpType.bypass,
                replica_groups=[list(range(NCORES))],
                ins=[chs[1][1].opt()], outs=[yos[1][1].opt()],
            )
            emit_gather(1, 1)
            emit_phase_c(1)

    nc.compile()
    return nc


def _host_reference(x, weights, cos, sin, mask, use_lora):
    """Numpy fallback for inputs outside the optimized assumptions."""
    (q_w, q_b, q_A, q_B, k_w, k_b, k_A, k_B,
     v_w, v_b, v_A, v_B, p_w, p_b, p_A, p_B) = weights

    def lin(xx, w, b, A, Bm):
        out = xx @ w.T + b
        if use_lora:
            out = out + (xx @ A) @ Bm
        return out

    def rope(t):
        x1, x2 = t[..., ::2], t[..., 1::2]
        y = np.stack((x1 * cos - x2 * sin, x1 * sin + x2 * cos), axis=-1)
        return y.reshape(t.shape)

    Bs, Tl, Cd = x.shape
    q = lin(x, q_w, q_b, q_A, q_B).reshape(Bs, Tl, H, D).transpose(0, 2, 1, 3)
    k = lin(x, k_w, k_b, k_A, k_B).reshape(Bs, Tl, H, D).transpose(0, 2, 1, 3)
    v = lin(x, v_w, v_b, v_A, v_B).reshape(Bs, Tl, H, D).transpose(0, 2, 1, 3)
    q, k = rope(q), rope(k)
    s = np.einsum('bhqd,bhkd->bhqk', q, k) / np.sqrt(D)
    s = np.where(mask, s, -np.inf)
    s = s - s.max(axis=-1, keepdims=True)
    p = np.exp(s)
    p /= p.sum(axis=-1, keepdims=True)
    o = np.einsum('bhqk,bhkd->bhqd', p, v).transpose(0, 2, 1, 3).reshape(Bs, Tl, Cd)
    return lin(o, p_w, p_b, p_A, p_B).astype(np.float32)


def _blk(a2d, parts=P):
    """[C, N] row-chunked -> [P, KC*N] with per-partition contiguous runs."""
    Cr, N = a2d.shape
    return np.ascontiguousarray(
        a2d.reshape(Cr // parts, parts, N).transpose(1, 0, 2).reshape(parts, -1))


def kernel(**inputs):
    x = np.asarray(inputs["x"], np.float32)
    cos = np.asarray(inputs["cos"], np.float32)
    sin = np.asarray(inputs["sin"], np.float32)
    mask = np.asarray(inputs["mask"])
    use_lora = int(np.asarray(inputs["use_lora"]))
    ws = {}
    for nm in ("q", "k", "v", "p"):
        for suf in ("w", "b", "A", "B"):
            ws[f"{nm}_{suf}"] = np.asarray(inputs[f"{nm}_{suf}"], np.float32)

    causal = bool((mask == np.tril(np.ones((T, T), bool))).all())
    zero_bias = all(not ws[f"{nm}_b"].any() for nm in ("q", "k", "v", "p"))
    if not (causal and zero_bias and x.shape == (B, T, C)):
        weights = tuple(ws[f"{nm}_{suf}"] for nm in ("q", "k", "v", "p")
                        for suf in ("w", "b", "A", "B"))
        return _host_reference(x, weights, cos, sin, mask, use_lora)

    # effective (LoRA-folded) transposed weights: out = x @ W_eff.T,
    # W_eff.T = w.T + A @ B
    effT = {}
    for nm in ("q", "k", "v", "p"):
        wt = ws[f"{nm}_w"].T.copy()
        if use_lora:
            wt += ws[f"{nm}_A"] @ ws[f"{nm}_B"]
        effT[nm] = np.ascontiguousarray(wt, np.float32)

    xT = np.ascontiguousarray(x.reshape(B * T, C).T)

    f8 = ml_dtypes.float8_e4m3
    bf = ml_dtypes.bfloat16

    def to8(a, s):
        return np.clip(np.asarray(a, np.float32) * s, -240.0, 240.0).astype(f8)

    # x, tile-blocked: [TT, P, KC*512]
    x8b = np.stack([_blk(to8(xT[:, t * 512:(t + 1) * 512], XS))
                    for t in range(TT)])
    xbb = np.stack([_blk(xT[:, 0:512].astype(bf)),
                    _blk(xT[:, T:T + 512].astype(bf))])

    # sigma: within each head reorder out-features to [evens, odds] so the
    # rope pair-rotation becomes a partition half-swap
    perm = np.concatenate([np.arange(0, D, 2), np.arange(1, D, 2)])
    cosT = cos.T.astype(np.float32)          # [64, T]
    sinT = sin.T.astype(np.float32)
    cosA = np.tile(np.vstack([cosT, cosT]), (1, B))          # [128, B*T]
    sinA = np.tile(np.vstack([-sinT, sinT]), (1, B))
    cosAs = cosA / (XS * WS)
    sinAs = sinA / (XS * WS)

    # additive causal mask factorization: M_o = lstep.T @ rmask_o where
    # M_o[j, q] = -1e9 iff j + 128*o > q (adds to scores before exp -> 0)
    lstep = np.tril(np.ones((P, P), np.float32)).T
    rmask = np.zeros((4, P, 512), np.float32)
    for o in range(4):
        for qr in range(512):
            m = max(0, qr + 1 - 128 * o)     # m=0 row covers fully-masked cols
            if m < P:
                rmask[o, m, qr] = -1e9

    # output projection weight, blocked [co, p, kcn, m]; contraction chunk
    # kcn = hl*8 + r maps to global row block 2r+hl (A2A delivery order)
    pwB = np.ascontiguousarray(
        effT["p"].reshape(KC, P, KC, P).transpose(2, 1, 0, 3))
    ordx = [2 * (j % 8) + (j // 8) for j in range(KC)]
    pwB = np.ascontiguousarray(pwB[:, :, ordx, :]).astype(bf)

    global _PROGRAM
    if _PROGRAM is None:
        _PROGRAM = _build_program()
    nc = _PROGRAM

    in_maps = []
    for c in range(NCORES):
        cols = slice(c * HPC * D, (c + 1) * HPC * D)
        wqT = effT["q"][:, cols].copy()
        wkT = effT["k"][:, cols].copy()
        for hl in range(HPC):
            sl = slice(hl * D, (hl + 1) * D)
            wqT[:, sl] = wqT[:, sl][:, perm]
            wkT[:, sl] = wkT[:, sl][:, perm]
        wvT = np.ascontiguousarray(effT["v"][:, cols])
        in_maps.append({
            "x8b": x8b,
            "xbb": xbb,
            "w8q": _blk(to8(wqT, WS)),
            "w8k": _blk(to8(wkT, WS)),
            "w8v": _blk(to8(wvT, WS)),
            "wbq": _blk(wqT.astype(bf)),
            "wbk": _blk(wkT.astype(bf)),
            "wbv": _blk(wvT.astype(bf)),
            "pwB": pwB,
            "cosA": cosA,
            "sinA": sinA,
            "cosAs": cosAs,
            "sinAs": sinAs,
            "lstep": lstep.astype(bf),
            "rmask": rmask.astype(bf),
        })

    res = run_bass_kernel_spmd(nc, in_maps, list(range(NCORES)))

    out = np.empty((B * T, C), np.float32)
    for c in range(NCORES):
        oT = np.asarray(res.results[c]["outT"], np.float32)        # [2048, 512]
        out[c * 256:(c + 1) * 256, :] = oT[:, 0:256].T             # b = 0
        out[T + c * 256:T + (c + 1) * 256, :] = oT[:, 256:512].T   # b = 1
    return out.reshape(B, T, C)


# revision 49
# speedup vs baseline: 1.0939x; 1.0939x over previous
"""Trainium2 Bass kernel for nn_Attention_35588099015470.

Full transformer attention block: LoRA linears (folded host-side) + RoPE +
causal SDPA + output projection, B=2 T=2048 C=2048 H=16 D=128, fp32 in/out.

Sharding: tensor-parallel over heads - 8 cores x 2 heads, AllToAll to
token-parallel for the output projection. Mixed-precision datapath:

 - q/k/v projections run in fp8e4m3 with DoubleRow perf mode (two 128-deep
   contraction chunks per pass = 2x MAC throughput), EXCEPT the first 512
   tokens of each batch which run in bf16: those tokens have small attention
   fan-in, so quantization errors there don't average out and dominate the
   max-error metric.
 - scores (QK^T) are bf16 (fp8 DR doesn't help at D=128 contraction).
 - softmax probabilities and PV run fp8+DoubleRow for query tiles >= 1;
   query tile 0 (tokens 0-511) runs bf16 against a bf16 copy of v.
 - output projection is bf16 (y values for early tokens are large and fp8
   there busts the error budget), split per batch: batch 0 projects in the
   shadow of the last attention block, batch 1 in two contraction stages so
   its first half overlaps the final AllToAll.
 - q/k/v activations stay SBUF-resident between phases, collectives and the
   projection weights are bf16, projection weights prefetch during attention,
   and all host tensors are tile-blocked so DMA moves 4-16KB per partition
   per transfer instead of 512B packets.
 - softmax row sums come from a ones-matmul folded into the fp8/bf16 PV
   stream; normalization is per-query-tile and eager so each AllToAll
   launches as soon as its head finishes.

Scale bookkeeping: x is scaled by XS=32 and weights by WS=2048 before fp8
quantization (keeps values clear of subnormals, max < 240); the 1/(XS*WS)
descale folds into the RoPE cos/sin tables for q/k and into the PSUM->SBUF
copy for v (which carries an extra VS=16 so fp8 v has headroom). exp() gets
a -ln(8) bias so unnormalized probabilities stay < 240 (fp8 max); the
normalization cancels both the bias and VS (the row-sum ones matmul uses VS
as its constant value).

Biases are guaranteed zero by the problem's setup_inputs and the mask is the
causal tril; if either assumption is violated at runtime we fall back to a
host reference implementation so the kernel stays correct on any input.
"""
import sys

sys.path.insert(0, "/opt/trn_rl_repo")

import numpy as np
import ml_dtypes
from contextlib import ExitStack

import concourse.tile as tile
from concourse import bacc, mybir
from concourse.bass_utils import run_bass_kernel_spmd

dt = mybir.dt
F8 = dt.float8e4
BF = dt.bfloat16
DR = mybir.MatmulPerfMode.DoubleRow

B, T, C, H, R = 2, 2048, 2048, 16, 8
D = C // H            # 128
NCORES = 8
HPC = H // NCORES     # heads per core = 2
P = 128
TT = (B * T) // 512   # 8 token tiles of 512
KC = C // P           # 16 contraction chunks
QT = T // 512         # 4 query tiles per (b, h)
SCALE = 1.0 / float(np.sqrt(D))

XS = 32.0             # x fp8 scale
WS = 2048.0           # weight fp8 scale
VS = 16.0             # v fp8 scale
EXP_BIAS = float(np.log(0.125))   # keeps exp output < 240 (fp8 max)
A8SC = VS / (XS * WS)             # fp8 v psum -> sbuf copy scale

BF_TT = (0, 4)        # token tiles computed in bf16 (first 512 tokens/batch)
TT_ORDER = [1, 2, 3, 5, 6, 7, 0, 4]   # fp8 tiles first (smaller first DMAs)

_PROGRAM = None


def _build_program():
    nc = bacc.Bacc("TRN2", target_bir_lowering=False, debug=False,
                   num_devices=NCORES)

    # tile-blocked inputs: [..., P, free] with per-partition-contiguous runs
    x8_d = nc.dram_tensor("x8b", [TT, P, KC * 512], F8, kind="ExternalInput")
    xb_d = nc.dram_tensor("xbb", [B, P, KC * 512], BF, kind="ExternalInput")
    w8_d = {nm: nc.dram_tensor(f"w8{nm}", [P, KC * HPC * D], F8,
                               kind="ExternalInput") for nm in ("q", "k", "v")}
    wb_d = {nm: nc.dram_tensor(f"wb{nm}", [P, KC * HPC * D], BF,
                               kind="ExternalInput") for nm in ("q", "k", "v")}
    pwB_d = nc.dram_tensor("pwB", [KC, P, KC, P], BF, kind="ExternalInput")
    cosA_d = nc.dram_tensor("cosA", [P, B * T], dt.float32, kind="ExternalInput")
    sinA_d = nc.dram_tensor("sinA", [P, B * T], dt.float32, kind="ExternalInput")
    cosAs_d = nc.dram_tensor("cosAs", [P, B * T], dt.float32, kind="ExternalInput")
    sinAs_d = nc.dram_tensor("sinAs", [P, B * T], dt.float32, kind="ExternalInput")
    lstep_d = nc.dram_tensor("lstep", [P, P], BF, kind="ExternalInput")
    rmask_d = nc.dram_tensor("rmask", [4, P, 512], BF, kind="ExternalInput")

    outT_d = nc.dram_tensor("outT", [C, 512], BF, kind="ExternalOutput")

    with tile.TileContext(nc) as tc, ExitStack() as ctx:
        dram = ctx.enter_context(tc.tile_pool(name="dram", bufs=1, space="DRAM"))
        # A2A staging: one collective per (batch, head-local), bf16
        chs = [[dram.tile([NCORES, D, 256], BF, name=f"ch_{b}_{hl}")
                for hl in range(HPC)] for b in range(B)]
        yos = [[dram.tile([NCORES * D, 256], BF, name=f"yo_{b}_{hl}")
                for hl in range(HPC)] for b in range(B)]

        # persistent SBUF (survives all phases)
        cst = ctx.enter_context(tc.tile_pool(name="cst", bufs=1))
        qT_sb = cst.tile([P, HPC, B * T], BF, name="qT_sb")
        kT_sb = cst.tile([P, HPC, B * T], BF, name="kT_sb")
        v8_sb = cst.tile([P, TT * 4, HPC * D], F8, name="v8_sb")
        v0_sb = cst.tile([P, B * 4, HPC * D], BF, name="v0_sb")

        ones_bf = cst.tile([P, P], BF, name="ones_bf")
        nc.any.memset(ones_bf[:], VS)
        ones8 = cst.tile([P, 2, P], F8, name="ones8")
        nc.any.memset(ones8[:], VS)
        lstep = cst.tile([P, P], BF, name="lstep")
        rmask = cst.tile([P, 4, 512], BF, name="rmask")
        ebias = cst.tile([P, 1], dt.float32, name="ebias")
        nc.any.memset(ebias[:], EXP_BIAS)

        # output-projection weights, prefetched in two halves: pw_lo during
        # phase A's DMA slack, pw_hi right after the first A2A's writes
        pwp = ctx.enter_context(tc.tile_pool(name="pw", bufs=1))
        pw_lo = pwp.tile([P, KC // 2, KC, P], BF, name="pw_lo")
        pw_hi = pwp.tile([P, KC // 2, KC, P], BF, name="pw_hi")

        # ---------------- Phase A: q/k/v projections + RoPE -----------------
        with tc.tile_pool(name="pa_w", bufs=1) as wp, \
             tc.tile_pool(name="pa_x8", bufs=3) as x8p, \
             tc.tile_pool(name="pa_xb", bufs=1) as xbp, \
             tc.tile_pool(name="pa_cs", bufs=2) as csp, \
             tc.tile_pool(name="pa_tmp", bufs=2) as tp, \
             tc.tile_pool(name="pa_ps", bufs=1, space="PSUM") as pp:

            # DMA emission order = queue processing order: first fp8 tile and
            # fp8 weights first (each split 4 ways across queues), then the
            # second tile, then the big bf16 sets
            def load4(dst3, src2, inner):
                # dst3 [P, KC, inner] <- src2 [P, KC*inner], 4 parallel DMAs
                for g in range(4):
                    nc.sync.dma_start(
                        dst3[:, g * 4:(g + 1) * 4, :],
                        src2[:, g * 4 * inner:(g + 1) * 4 * inner])

            xts = {}
            xt = x8p.tile([P, KC, 512], F8, name=f"xt8_{TT_ORDER[0]}", tag="x8")
            load4(xt, x8_d.ap()[TT_ORDER[0]], 512)
            xts[TT_ORDER[0]] = xt
            w8s = {}
            for nm in ("q", "k", "v"):
                w_sb = wp.tile([P, KC, HPC * D], F8, name=f"w8{nm}_sb")
                load4(w_sb, w8_d[nm].ap(), HPC * D)
                w8s[nm] = w_sb
            xt = x8p.tile([P, KC, 512], F8, name=f"xt8_{TT_ORDER[1]}", tag="x8")
            load4(xt, x8_d.ap()[TT_ORDER[1]], 512)
            xts[TT_ORDER[1]] = xt
            cs_first = {}
            tt0 = TT_ORDER[0]
            for tag, dsrc in (("csc", cosAs_d), ("css", sinAs_d)):
                t_ = csp.tile([P, 512], dt.float32, tag=tag, name=f"{tag}_{tt0}")
                nc.sync.dma_start(t_[:], dsrc.ap()[:, tt0 * 512:(tt0 + 1) * 512])
                cs_first[tag] = t_
            wbs = {}
            for nm in ("q", "k", "v"):
                w_sb = wp.tile([P, KC, HPC * D], BF, name=f"wb{nm}_sb")
                nc.sync.dma_start(w_sb[:], wb_d[nm].ap())
                wbs[nm] = w_sb
            nc.sync.dma_start(lstep[:], lstep_d.ap())
            for o in range(4):
                nc.sync.dma_start(rmask[:, o, :], rmask_d.ap()[o])

            xbts = {}
            for ti, tt in enumerate(TT_ORDER):
                tsl = slice(tt * 512, (tt + 1) * 512)
                bf = tt in BF_TT
                if ti == 1:
                    # bf16 x for tile 0 early, while tiles 1-2 compute
                    xb0 = xbp.tile([P, KC, 512], BF, name="xtb_0", tag="xb")
                    for g in range(8):
                        nc.sync.dma_start(
                            xb0[:, g * 2:(g + 1) * 2, :],
                            xb_d.ap()[0][:, g * 1024:(g + 1) * 1024])
                    xbts[0] = xb0
                if ti == 3:
                    # prefetch the projection weights in phase A's DMA slack
                    # (keeping them out of phase B's queues, where they would
                    # delay the A2A staging writes via coarse DMA semaphores)
                    for co in range(KC // 2):
                        nc.sync.dma_start(pw_lo[:, co, :, :], pwB_d.ap()[co])
                if ti == 4:
                    xb4 = xbp.tile([P, KC, 512], BF, name="xtb_4", tag="xb")
                    load4(xb4, xb_d.ap()[1], 512)
                    xbts[4] = xb4
                if ti == 5:
                    for co in range(KC // 2):
                        nc.sync.dma_start(pw_hi[:, co, :, :],
                                          pwB_d.ap()[KC // 2 + co])
                # fp8 x tiles prefetch two iterations ahead so the DMA
                # queue has a full tile of lead time
                if ti + 2 < len(TT_ORDER):
                    nxt = TT_ORDER[ti + 2]
                    if nxt not in BF_TT and nxt not in xts:
                        xt2 = x8p.tile([P, KC, 512], F8, name=f"xt8_{nxt}",
                                       tag="x8")
                        load4(xt2, x8_d.ap()[nxt], 512)
                        xts[nxt] = xt2
                if bf:
                    xt = xbts[tt]
                else:
                    xt = xts[tt]
                cd, sd = (cosA_d, sinA_d) if bf else (cosAs_d, sinAs_d)
                if ti == 0:
                    cs_c, cs_s = cs_first["csc"], cs_first["css"]
                else:
                    cs_c = csp.tile([P, 512], dt.float32, tag="csc",
                                    name=f"csc_{tt}")
                    nc.sync.dma_start(cs_c[:], cd.ap()[:, tsl])
                    cs_s = csp.tile([P, 512], dt.float32, tag="css",
                                    name=f"css_{tt}")
                    nc.sync.dma_start(cs_s[:], sd.ap()[:, tsl])

                for nm, dst in (("q", qT_sb), ("k", kT_sb)):
                    for mt in range(HPC):
                        ps = pp.tile([P, 512], dt.float32, tag="qk", bufs=4,
                                     name=f"psA_{tt}_{nm}_{mt}")
                        msl = slice(mt * P, (mt + 1) * P)
                        if bf:
                            w_sb = wbs[nm]
                            for kc in range(KC):
                                nc.tensor.matmul(
                                    ps[:], w_sb[:, kc, msl], xt[:, kc, :],
                                    start=(kc == 0), stop=(kc == KC - 1))
                        else:
                            w_sb = w8s[nm]
                            for c in range(KC // 2):
                                nc.tensor.matmul(
                                    ps[:], w_sb[:, 2 * c:2 * c + 2, msl],
                                    xt[:, 2 * c:2 * c + 2, :],
                                    start=(c == 0), stop=(c == KC // 2 - 1),
                                    perf_mode=DR)
                        # rope: y = raw*cosA + halfswap(raw)*sinA
                        # (cs tiles carry the fp8 descale for fp8 tiles)
                        t1 = tp.tile([P, 512], dt.float32, tag="t1",
                                     name=f"t1_{tt}_{nm}_{mt}")
                        nc.vector.tensor_mul(t1[:], ps[:], cs_c[:])
                        t2 = tp.tile([P, 512], dt.float32, tag="t2",
                                     name=f"t2_{tt}_{nm}_{mt}")
                        nc.vector.tensor_mul(t2[0:64, :], ps[64:128, :],
                                             cs_s[0:64, :])
                        nc.vector.tensor_mul(t2[64:128, :], ps[0:64, :],
                                             cs_s[64:128, :])
                        nc.vector.tensor_add(dst[:, mt, tsl], t1[:], t2[:])

                # v in natural [token, d] layout: x chunks stationary,
                # weight chunks moving -> out [128 tokens, 256 features]
                for tc_ in range(4):
                    psv = pp.tile([P, HPC * D], dt.float32, tag="vv", bufs=4,
                                  name=f"psV_{tt}_{tc_}")
                    tcs = slice(tc_ * P, (tc_ + 1) * P)
                    if bf:
                        w_sb = wbs["v"]
                        for kc in range(KC):
                            nc.tensor.matmul(
                                psv[:], xt[:, kc, tcs], w_sb[:, kc, :],
                                start=(kc == 0), stop=(kc == KC - 1))
                    else:
                        w_sb = w8s["v"]
                        for c in range(KC // 2):
                            nc.tensor.matmul(
                                psv[:], xt[:, 2 * c:2 * c + 2, tcs],
                                w_sb[:, 2 * c:2 * c + 2, :],
                                start=(c == 0), stop=(c == KC // 2 - 1),
                                perf_mode=DR)
                    sc = VS if bf else A8SC
                    nc.scalar.activation(v8_sb[:, tt * 4 + tc_, :], psv[:],
                                         mybir.ActivationFunctionType.Copy,
                                         scale=sc)
                    if bf:
                        nc.scalar.activation(
                            v0_sb[:, (tt // 4) * 4 + tc_, :], psv[:],
                            mybir.ActivationFunctionType.Copy, scale=VS)

        # ---------------- Phase B: causal attention per (b, head) ----------
        # + batch-0 output projection in the shadow of the last block
        ycp = ctx.enter_context(tc.tile_pool(name="yc", bufs=1))
        ocp = ctx.enter_context(tc.tile_pool(name="oc", bufs=3))

        def pw_co(co):
            return pw_lo[:, co, :, :] if co < KC // 2 else \
                pw_hi[:, co - KC // 2, :, :]

        ybs = {0: {}, 1: {}}

        def emit_gather(b, hl):
            yb = ycp.tile([P, NCORES, 256], BF, name=f"yb_{b}_{hl}")
            nc.sync.dma_start(
                yb[:], yos[b][hl][:].rearrange("(a p) t -> p a t", p=P))
            ybs[b][hl] = yb

        with tc.tile_pool(name="pb_p8", bufs=4) as p8p, \
             tc.tile_pool(name="pb_pb", bufs=3) as pbp, \
             tc.tile_pool(name="pb_pv", bufs=6) as pvp, \
             tc.tile_pool(name="pb_y", bufs=8) as yp, \
             tc.tile_pool(name="pb_ps", bufs=1, space="PSUM") as pb:

            for b in range(B):
                for hl in range(HPC):
                    kT_h = kT_sb[:, hl, b * T:(b + 1) * T]
                    qT_h = qT_sb[:, hl, b * T:(b + 1) * T]
                    hsl = slice(hl * D, (hl + 1) * D)

                    for qt in range(QT):
                        qTt = qT_h[:, qt * 512:(qt + 1) * 512]
                        n = 4 * (qt + 1)
                        smps = pb.tile([P, 512], dt.float32, tag="sm", bufs=1,
                                       name=f"sm_{b}_{hl}_{qt}")
                        pvps = pb.tile([P, 512], dt.float32, tag="pv", bufs=2,
                                       name=f"pv_{b}_{hl}_{qt}")

                        sc_tiles = {}
                        npair = n // 2

                        def emit_sc_pair(c, _q=qTt, _k=kT_h, _n=n,
                                         _sc=sc_tiles, _b=b, _hl=hl, _qt=qt):
                            # score pair in one 2-bank psum tile so one exp
                            # covers both chunks (ACT has ~300ns fixed cost)
                            ps = pb.tile([P, 2, 512], dt.float32, tag="sc",
                                         bufs=2, name=f"sc_{_b}_{_hl}_{_qt}_{c}")
                            for half in range(2):
                                jc = 2 * c + half
                                diag = jc >= _n - 4
                                nc.tensor.matmul(
                                    ps[:, half, :],
                                    _k[:, jc * P:(jc + 1) * P], _q[:],
                                    start=True, stop=not diag)
                                if diag:
                                    o = jc - (_n - 4)
                                    nc.tensor.matmul(ps[:, half, :], lstep[:],
                                                     rmask[:, o, :],
                                                     start=False, stop=True)
                            _sc[c] = ps

                        emit_sc_pair(0)
                        if npair > 1:
                            emit_sc_pair(1)
                        for c in range(npair):
                            scps = sc_tiles.pop(c)
                            if qt == 0:
                                # bf16 path: accurate probabilities and v for
                                # the low-fan-in early tokens
                                pT = pbp.tile([P, 2, 512], BF, tag="pTb",
                                              name=f"pTb_{b}_{hl}_{c}")
                            else:
                                pT = p8p.tile([P, 2, 512], F8, tag="pT8",
                                              name=f"pT8_{b}_{hl}_{qt}_{c}")
                            nc.scalar.activation(
                                pT[:], scps[:],
                                mybir.ActivationFunctionType.Exp,
                                bias=ebias[:], scale=SCALE)
                            if c + 2 < npair:
                                emit_sc_pair(c + 2)
                            if qt == 0:
                                for half in range(2):
                                    jc = 2 * c + half
                                    nc.tensor.matmul(
                                        smps[:], ones_bf[:], pT[:, half, :],
                                        start=(jc == 0), stop=(jc == 3))
                                    nc.tensor.matmul(
                                        pvps[:], v0_sb[:, b * 4 + jc, hsl],
                                        pT[:, half, :],
                                        start=(jc == 0), stop=(jc == 3))
                            else:
                                nc.tensor.matmul(
                                    smps[:], ones8[:], pT[:],
                                    start=(c == 0), stop=(c == npair - 1),
                                    perf_mode=DR)
                                nc.tensor.matmul(
                                    pvps[:],
                                    v8_sb[:, b * 16 + 2 * c:b * 16 + 2 * c + 2,
                                          hsl],
                                    pT[:],
                                    start=(c == 0), stop=(c == npair - 1),
                                    perf_mode=DR)

                        # eager per-qt normalization: fast-approx reciprocal
                        # of the sum row, broadcast, scale PV
                        pv_sb = pvp.tile([P, 512], dt.float32, tag="pvsb",
                                         name=f"pvsb_{b}_{hl}_{qt}")
                        nc.vector.tensor_copy(pv_sb[:], pvps[:])
                        smrow = yp.tile([1, 512], dt.float32, tag="smrow",
                                        bufs=6, name=f"smrow_{b}_{hl}_{qt}")
                        nc.vector.tensor_copy(smrow[:], smps[0:1, :])
                        rrow = yp.tile([1, 512], dt.float32, tag="rrow",
                                       bufs=6, name=f"rrow_{b}_{hl}_{qt}")
                        rscr = yp.tile([1, 512], dt.float32, tag="rscr",
                                       bufs=6, name=f"rscr_{b}_{hl}_{qt}")
                        nc.vector.reciprocal_approx_accurate(
                            rrow[:], smrow[:], rscr[:])
                        bc = yp.tile([P, 512], dt.float32, tag="bc", bufs=4,
                                     name=f"bc_{b}_{hl}_{qt}")
                        nc.gpsimd.partition_broadcast(bc[:], rrow[:])
                        yt = yp.tile([P, 512], BF, tag="yt",
                                     name=f"yt_{b}_{hl}_{qt}")
                        nc.vector.tensor_mul(yt[:], pv_sb[:], bc[:])
                        nc.sync.dma_start(chs[b][hl][2 * qt][:, :],
                                          yt[:, 0:256])
                        nc.sync.dma_start(chs[b][hl][2 * qt + 1][:, :],
                                          yt[:, 256:512])

                    if not (b == 1 and hl == 1):
                        nc.gpsimd.collective_compute(
                            "AllToAll", mybir.AluOpType.bypass,
                            replica_groups=[list(range(NCORES))],
                            ins=[chs[b][hl].opt()], outs=[yos[b][hl].opt()],
                        )
                    if b == 1 and hl == 0:
                        emit_gather(1, 0)
                        emit_gather(0, 0)
                        emit_gather(0, 1)

        # ------------ Phase C: output projection, both batches -------------
        # all four attention blocks ran back-to-back; batch 0's data is long
        # delivered, so C-b0 runs while the (1,1) A2A (emitted below,
        # triggered from gpsimd as soon as its staging writes land) flies.
        # C-b1's first matmul is emitted after the (1,1) gather, so its
        # coarse DMA-semaphore wait covers exactly that gather. Phase C gets
        # its own PSUM pool (attention's is closed) so four accumulators can
        # pipeline the evacuations.
        with tc.tile_pool(name="pc_ps", bufs=1, space="PSUM") as pc:

            def emit_phase_c(b):
                for co in range(KC):
                    pso = pc.tile([P, 256], dt.float32, tag="co", bufs=4,
                                  name=f"pso_{b}_{co}")
                    i = 0
                    for hl in range(HPC):
                        for r in range(NCORES):
                            nc.tensor.matmul(
                                pso[:], pw_co(co)[:, hl * 8 + r, :],
                                ybs[b][hl][:, r, :],
                                start=(i == 0), stop=(i == KC - 1))
                            i += 1
                    oo = ocp.tile([P, 256], BF, tag="oo",
                                  name=f"oo_{b}_{co}")
                    nc.scalar.copy(oo[:], pso[:])
                    nc.sync.dma_start(
                        outT_d.ap()[co * P:(co + 1) * P, b * 256:(b + 1) * 256],
                        oo[:])

            emit_phase_c(0)
            nc.gpsimd.collective_compute(
                "AllToAll", mybir.AluOpType.bypass,
                replica_groups=[list(range(NCORES))],
                ins=[chs[1][1].opt()], outs=[yos[1][1].opt()],
            )
            emit_gather(1, 1)
            emit_phase_c(1)

    nc.compile()
    return nc


def _host_reference(x, weights, cos, sin, mask, use_lora):
    """Numpy fallback for inputs outside the optimized assumptions."""
    (q_w, q_b, q_A, q_B, k_w, k_b, k_A, k_B,
     v_w, v_b, v_A, v_B, p_w, p_b, p_A, p_B) = weights

    def lin(xx, w, b, A, Bm):
        out = xx @ w.T + b
        if use_lora:
            out = out + (xx @ A) @ Bm
        return out

    def rope(t):
        x1, x2 = t[..., ::2], t[..., 1::2]
        y = np.stack((x1 * cos - x2 * sin, x1 * sin + x2 * cos), axis=-1)
        return y.reshape(t.shape)

    Bs, Tl, Cd = x.shape
    q = lin(x, q_w, q_b, q_A, q_B).reshape(Bs, Tl, H, D).transpose(0, 2, 1, 3)
    k = lin(x, k_w, k_b, k_A, k_B).reshape(Bs, Tl, H, D).transpose(0, 2, 1, 3)
    v = lin(x, v_w, v_b, v_A, v_B).reshape(Bs, Tl, H, D).transpose(0, 2, 1, 3)
    q, k = rope(q), rope(k)
    s = np.einsum('bhqd,bhkd->bhqk', q, k) / np.sqrt(D)
    s = np.where(mask, s, -np.inf)
    s = s - s.max(axis=-1, keepdims=True)
    p = np.exp(s)
    p /= p.sum(axis=-1, keepdims=True)
    o = np.einsum('bhqk,bhkd->bhqd', p, v).transpose(0, 2, 1, 3).reshape(Bs, Tl, Cd)
    return lin(o, p_w, p_b, p_A, p_B).astype(np.float32)


def _blk(a2d, parts=P):
    """[C, N] row-chunked -> [P, KC*N] with per-partition contiguous runs."""
    Cr, N = a2d.shape
    return np.ascontiguousarray(
        a2d.reshape(Cr // parts, parts, N).transpose(1, 0, 2).reshape(parts, -1))


def kernel(**inputs):
    x = np.asarray(inputs["x"], np.float32)
    cos = np.asarray(inputs["cos"], np.float32)
    sin = np.asarray(inputs["sin"], np.float32)
    mask = np.asarray(inputs["mask"])
    use_lora = int(np.asarray(inputs["use_lora"]))
    ws = {}
    for nm in ("q", "k", "v", "p"):
        for suf in ("w", "b", "A", "B"):
            ws[f"{nm}_{suf}"] = np.asarray(inputs[f"{nm}_{suf}"], np.float32)

    causal = bool((mask == np.tril(np.ones((T, T), bool))).all())
    zero_bias = all(not ws[f"{nm}_b"].any() for nm in ("q", "k", "v", "p"))
    if not (causal and zero_bias and x.shape == (B, T, C)):
        weights = tuple(ws[f"{nm}_{suf}"] for nm in ("q", "k", "v", "p")
                        for suf in ("w", "b", "A", "B"))
        return _host_reference(x, weights, cos, sin, mask, use_lora)

    # effective (LoRA-folded) transposed weights: out = x @ W_eff.T,
    # W_eff.T = w.T + A @ B
    effT = {}
    for nm in ("q", "k", "v", "p"):
        wt = ws[f"{nm}_w"].T.copy()
        if use_lora:
            wt += ws[f"{nm}_A"] @ ws[f"{nm}_B"]
        effT[nm] = np.ascontiguousarray(wt, np.float32)

    xT = np.ascontiguousarray(x.reshape(B * T, C).T)

    f8 = ml_dtypes.float8_e4m3
    bf = ml_dtypes.bfloat16

    def to8(a, s):
        return np.clip(np.asarray(a, np.float32) * s, -240.0, 240.0).astype(f8)

    # x, tile-blocked: [TT, P, KC*512]
    x8b = np.stack([_blk(to8(xT[:, t * 512:(t + 1) * 512], XS))
                    for t in range(TT)])
    xbb = np.stack([_blk(xT[:, 0:512].astype(bf)),
                    _blk(xT[:, T:T + 512].astype(bf))])

    # sigma: within each head reorder out-features to [evens, odds] so the
    # rope pair-rotation becomes a partition half-swap
    perm = np.concatenate([np.arange(0, D, 2), np.arange(1, D, 2)])
    cosT = cos.T.astype(np.float32)          # [64, T]
    sinT = sin.T.astype(np.float32)
    cosA = np.tile(np.vstack([cosT, cosT]), (1, B))          # [128, B*T]
    sinA = np.tile(np.vstack([-sinT, sinT]), (1, B))
    cosAs = cosA / (XS * WS)
    sinAs = sinA / (XS * WS)

    # additive causal mask factorization: M_o = lstep.T @ rmask_o where
    # M_o[j, q] = -1e9 iff j + 128*o > q (adds to scores before exp -> 0)
    lstep = np.tril(np.ones((P, P), np.float32)).T
    rmask = np.zeros((4, P, 512), np.float32)
    for o in range(4):
        for qr in range(512):
            m = max(0, qr + 1 - 128 * o)     # m=0 row covers fully-masked cols
            if m < P:
                rmask[o, m, qr] = -1e9

    # output projection weight, blocked [co, p, kcn, m]; contraction chunk
    # kcn = hl*8 + r maps to global row block 2r+hl (A2A delivery order)
    pwB = np.ascontiguousarray(
        effT["p"].reshape(KC, P, KC, P).transpose(2, 1, 0, 3))
    ordx = [2 * (j % 8) + (j // 8) for j in range(KC)]
    pwB = np.ascontiguousarray(pwB[:, :, ordx, :]).astype(bf)

    global _PROGRAM
    if _PROGRAM is None:
        _PROGRAM = _build_program()
    nc = _PROGRAM

    in_maps = []
    for c in range(NCORES):
        cols = slice(c * HPC * D, (c + 1) * HPC * D)
        wqT = effT["q"][:, cols].copy()
        wkT = effT["k"][:, cols].copy()
        for hl in range(HPC):
            sl = slice(hl * D, (hl + 1) * D)
            wqT[:, sl] = wqT[:, sl][:, perm]
            wkT[:, sl] = wkT[:, sl][:, perm]
        wvT = np.ascontiguousarray(effT["v"][:, cols])
        in_maps.append({
            "x8b": x8b,
            "xbb": xbb,
            "w8q": _blk(to8(wqT, WS)),
            "w8k": _blk(to8(wkT, WS)),
            "w8v": _blk(to8(wvT, WS)),
            "wbq": _blk(wqT.astype(bf)),
            "wbk": _blk(wkT.astype(bf)),
            "wbv": _blk(wvT.astype(bf)),
            "pwB": pwB,
            "cosA": cosA,
            "sinA": sinA,
            "cosAs": cosAs,
            "sinAs": sinAs,
            "lstep": lstep.astype(bf),
            "rmask": rmask.astype(bf),
        })

    res = run_bass_kernel_spmd(nc, in_maps, list(range(NCORES)))

    out = np.empty((B * T, C), np.float32)
    for c in range(NCORES):
        oT = np.asarray(res.results[c]["outT"], np.float32)        # [2048, 512]
        out[c * 256:(c + 1) * 256, :] = oT[:, 0:256].T             # b = 0
        out[T + c * 256:T + (c + 1) * 256, :] = oT[:, 256:512].T   # b = 1
    return out.reshape(B, T, C)
